# revision 4
# baseline (speedup 1.0000x reference)
"""DefectAttractor (retrieval KNN) Trainium2 Bass kernel.

v7 (bitwise-exact Sign count + sqrt-free stats) plus software pipelining:
  - per-tile index extraction and gather issue, lagged 2 tiles behind the
    sign count so no engine stalls waiting cross-engine;
  - each chunk's epilogue (stats, dir, out-scale, out DMA) is emitted one
    chunk late, after the next chunk's front-end, so its instructions never
    block the scans/matmuls in any engine's FIFO;
  - fp16 output (upcast on host).
Front-end per tile: 8 fp16 MMs (PE) -> running-min scan (DVE) -> exact
sign count (ACT, bias = raw last-column view). maxv chained on DVE.
"""
import numpy as np
from contextlib import ExitStack

import concourse.bass as bass
import concourse.bacc as bacc
import concourse.tile as tile
import concourse.mybir as mybir
import concourse.bass_utils as bass_utils

N, M, D = 131072, 1024, 128
NCORES = 8
R = N // NCORES
P = 128
T = R // P
TCH = 32
NCHUNK = T // TCH
EPS = 1e-8
E = 192
POS_BIG = 1e30
LAG = 2                    # tiles between sign count and gather issue

f16 = mybir.dt.float16
f32 = mybir.dt.float32
i32 = mybir.dt.int32
Alu = mybir.AluOpType
Act = mybir.ActivationFunctionType

_cache = {}


def _build(rate, coh, tanf, repeat=1):
    assert coh >= 0.0 and tanf >= 0.0, "squared yield compare needs coh,tanf>=0"
    nc = bacc.Bacc("TRN2", target_bir_lowering=False, debug=False,
                   num_devices=NCORES)

    xh_d = nc.dram_tensor("xh_t", [P, R], f16, kind="ExternalInput")
    xl_d = nc.dram_tensor("xl_t", [P, R], f16, kind="ExternalInput")
    x_d = nc.dram_tensor("x_shuf", [P, R], f32, kind="ExternalInput")
    sh_d = nc.dram_tensor("nsh_t", [P, M], f16, kind="ExternalInput")
    sl_d = nc.dram_tensor("nsl_t", [P, M], f16, kind="ExternalInput")
    b3_d = nc.dram_tensor("nbias3", [4, M], f16, kind="ExternalInput")
    sa_d = nc.dram_tensor("s_aug", [M, E], f32, kind="ExternalInput")
    x2_d = nc.dram_tensor("x2in", [P, T], f32, kind="ExternalInput")
    x2s_d = nc.dram_tensor("x2s", [P, T], f32, kind="ExternalInput")
    xq_d = nc.dram_tensor("xq", [P, T], f32, kind="ExternalInput")
    out_d = nc.dram_tensor("out", [P, R], f16, kind="ExternalOutput")

    rate = np.float32(rate)
    n2r2 = float(np.float32(-2.0) * rate * rate)
    c05v = float(rate * np.float32(0.5))
    cdel = float(rate * np.float32(1.5))

    with tile.TileContext(nc) as tc, ExitStack() as ctx:
        const = ctx.enter_context(tc.tile_pool(name="const", bufs=1))
        xw = ctx.enter_context(tc.tile_pool(name="xw", bufs=2))
        xnat = ctx.enter_context(tc.tile_pool(name="xnat", bufs=2))
        zpool = ctx.enter_context(tc.tile_pool(name="zp", bufs=3, space="PSUM"))
        rpool = ctx.enter_context(tc.tile_pool(name="rp", bufs=4))
        junk = ctx.enter_context(tc.tile_pool(name="junk", bufs=3))
        stats = ctx.enter_context(tc.tile_pool(name="stats", bufs=3))
        gpool = ctx.enter_context(tc.tile_pool(name="gp", bufs=TCH + 6))
        dpool = ctx.enter_context(tc.tile_pool(name="dp", bufs=3))
        opool = ctx.enter_context(tc.tile_pool(name="op", bufs=2))

        shT = const.tile([P, M], f16)
        slT = const.tile([P, M], f16)
        bias3 = const.tile([4, M], f16)
        ones3 = const.tile([4, 1], f16)
        posb = const.tile([P, 1], f32)
        c05t = const.tile([P, 1], f32)
        zero = const.tile([P, 1], f32)
        cohb = const.tile([P, 1], f32)
        nc.sync.dma_start(shT[:], sh_d.ap())
        nc.sync.dma_start(slT[:], sl_d.ap())
        nc.sync.dma_start(bias3[:], b3_d.ap())
        nc.vector.memset(ones3[:], 1.0)
        nc.vector.memset(posb[:], POS_BIG)
        nc.vector.memset(c05t[:], c05v)
        nc.vector.memset(zero[:], 0.0)
        nc.vector.memset(cohb[:], float(coh))

        def front_end(ch):
            """Front-end of chunk ch; returns state dict for its epilogue."""
            st = {}
            ccols = slice(ch * TCH * P, (ch + 1) * TCH * P)
            st["ccols"] = ccols
            xh_c = xw.tile([P, TCH * P], f16, tag="xh")
            nc.sync.dma_start(xh_c[:], xh_d.ap()[:, ccols])
            xl_c = xw.tile([P, TCH * P], f16, tag="xl")
            nc.sync.dma_start(xl_c[:], xl_d.ap()[:, ccols])
            x_c = xnat.tile([P, TCH, P], f32, tag="xn")
            nc.sync.dma_start(
                x_c[:], x_d.ap()[:, ccols].rearrange("p (t d) -> p t d", d=P))
            st["x_c"] = x_c
            for nm, dd in (("x2", x2_d), ("x2s", x2s_d), ("xq", xq_d)):
                tcl = stats.tile([P, TCH], f32, tag=nm)
                nc.sync.dma_start(tcl[:], dd.ap()[:, ch * TCH:(ch + 1) * TCH])
                st[nm] = tcl
            maxv_c = stats.tile([P, TCH], f32, tag="maxv")
            sg_c = stats.tile([P, TCH], f32, tag="sgc")
            idxi = stats.tile([P, TCH], i32, tag="idxi")
            s2g_c = stats.tile([P, TCH], f32, tag="s2g")
            g_list = []
            st["maxv"], st["s2g"], st["g"] = maxv_c, s2g_c, g_list

            def issue_gather(tl):
                nc.vector.tensor_scalar(idxi[:, tl:tl + 1], sg_c[:, tl:tl + 1],
                                        -1.0, None, op0=Alu.mult)
                g = gpool.tile([P, E], f32, tag="g")
                nc.gpsimd.indirect_dma_start(
                    out=g[:], out_offset=None, in_=sa_d.ap(),
                    in_offset=bass.IndirectOffsetOnAxis(
                        ap=idxi[:, tl:tl + 1], axis=0))
                g_list.append(g)
                nc.gpsimd.tensor_copy(s2g_c[:, tl:tl + 1], g[:, D:D + 1])

            for tl in range(TCH):
                tcols = slice(tl * P, (tl + 1) * P)
                xh_t = xh_c[:, tcols]
                xl_t = xl_c[:, tcols]
                z = zpool.tile([P, M], f32, tag="z")
                b0 = slice(0, 512)
                b1 = slice(512, 1024)
                nc.tensor.matmul(z[:, b0], ones3[:].to_broadcast([4, P]),
                                 bias3[:, b0], start=True, stop=False)
                nc.tensor.matmul(z[:, b1], ones3[:].to_broadcast([4, P]),
                                 bias3[:, b1], start=True, stop=False)
                nc.tensor.matmul(z[:, b0], xh_t, shT[:, b0], start=False, stop=False)
                nc.tensor.matmul(z[:, b1], xh_t, shT[:, b1], start=False, stop=False)
                nc.tensor.matmul(z[:, b0], xh_t, slT[:, b0], start=False, stop=False)
                nc.tensor.matmul(z[:, b1], xh_t, slT[:, b1], start=False, stop=False)
                nc.tensor.matmul(z[:, b0], xl_t, shT[:, b0], start=False, stop=True)
                nc.tensor.matmul(z[:, b1], xl_t, shT[:, b1], start=False, stop=True)

                r = rpool.tile([P, M], f32, tag="r")
                nc.vector.tensor_tensor_scan(
                    r[:], z[:], posb[:].to_broadcast([P, M]), POS_BIG,
                    op0=Alu.min, op1=Alu.min)
                nc.vector.tensor_scalar(maxv_c[:, tl:tl + 1], r[:, M - 1:M],
                                        -1.0, None, op0=Alu.mult)
                jk = junk.tile([P, M], f16, tag="jk")
                nc.scalar.activation(jk[:], r[:], Act.Sign,
                                     bias=r[:, M - 1:M], scale=-1.0,
                                     accum_out=sg_c[:, tl:tl + 1])
                if tl >= LAG:
                    issue_gather(tl - LAG)
            for tl in range(TCH - LAG, TCH):
                issue_gather(tl)
            return st

        def epilogue(st):
            maxv_c, s2g_c, g_list, x_c = st["maxv"], st["s2g"], st["g"], st["x_c"]
            x2_c, x2s_c, xq_c = st["x2"], st["x2s"], st["xq"]
            lhs = stats.tile([P, TCH], f32, tag="lhs")
            nc.vector.scalar_tensor_tensor(
                lhs[:], maxv_c[:], n2r2, x2s_c[:], op0=Alu.mult, op1=Alu.add)
            t1 = stats.tile([P, TCH], f32, tag="t1")
            nc.vector.scalar_tensor_tensor(
                t1[:], s2g_c[:], 0.5, maxv_c[:], op0=Alu.mult, op1=Alu.add)
            w = stats.tile([P, TCH], f32, tag="w")
            nc.gpsimd.tensor_tensor(w[:], t1[:], x2_c[:], op=Alu.subtract)
            aw = stats.tile([P, TCH], f32, tag="aw")
            nc.scalar.activation(aw[:], w[:], Act.Abs, bias=zero[:])
            rhs = stats.tile([P, TCH], f32, tag="rhs")
            nc.gpsimd.tensor_tensor(rhs[:], aw[:], xq_c[:], op=Alu.mult)
            rhs2 = stats.tile([P, TCH], f32, tag="rhs2")
            nc.scalar.activation(rhs2[:], rhs[:], Act.Square, bias=cohb[:])
            exc = stats.tile([P, TCH], f32, tag="exc")
            nc.vector.tensor_tensor(exc[:], lhs[:], rhs2[:], op=Alu.is_gt)
            coef = stats.tile([P, TCH], f32, tag="coef")
            nc.vector.scalar_tensor_tensor(
                coef[:], exc[:], cdel, c05t[:].to_broadcast([P, TCH]),
                op0=Alu.mult, op1=Alu.add)
            out_ch = opool.tile([P, TCH, P], f16, tag="out")
            for tl in range(TCH):
                dir_t = dpool.tile([P, P], f32, tag="dir")
                nc.gpsimd.tensor_tensor(dir_t[:], g_list[tl][:, :D],
                                        x_c[:, tl, :], op=Alu.subtract)
                nc.scalar.activation(out_ch[:, tl, :], dir_t[:], Act.Copy,
                                     bias=0.0, scale=coef[:, tl:tl + 1])
            nc.sync.dma_start(
                out_d.ap()[:, st["ccols"]].rearrange("p (t d) -> p t d", d=P),
                out_ch[:])

        import contextlib
        loop_cm = tc.For_i(0, repeat, 1) if repeat > 1 else contextlib.nullcontext()
        with loop_cm:
            prev = None
            for ch in range(NCHUNK):
                st = front_end(ch)
                if prev is not None:
                    epilogue(prev)
                prev = st
            epilogue(prev)

    nc.compile()
    return nc


def _prep(x, s, rate, coh, tanf):
    xT = np.ascontiguousarray(x.T)
    xh = xT.astype(np.float16)
    xl = (xT - xh.astype(np.float32)).astype(np.float16)

    nsT = np.ascontiguousarray(-s.T)
    nsh = nsT.astype(np.float16)
    nsl = (nsT - nsh.astype(np.float32)).astype(np.float16)

    s2_64 = (s.astype(np.float64) ** 2).sum(1)
    nbias = 0.5 * s2_64
    b1 = nbias.astype(np.float16)
    b2 = (nbias - b1.astype(np.float64)).astype(np.float16)
    b3 = (nbias - b1.astype(np.float64) - b2.astype(np.float64)).astype(np.float16)
    nbias3 = np.zeros((4, M), np.float16)
    nbias3[0], nbias3[1], nbias3[2] = b1, b2, b3

    s_aug = np.zeros((M, E), np.float32)
    s_aug[:, :D] = s
    s_aug[:, D] = s2_64.astype(np.float32)

    x2_64 = (x.astype(np.float64) ** 2).sum(1)
    x2 = x2_64.astype(np.float32)[:, None]
    r64 = np.float64(np.float32(rate))
    x2s = (r64 * r64 * x2_64).astype(np.float32)[:, None]
    xq = (np.float64(np.float32(tanf)) * r64 /
          (np.sqrt(x2_64) + EPS)).astype(np.float32)[:, None]
    return {"xh": xh, "xl": xl, "x": x, "nsh": nsh, "nsl": nsl,
            "nbias3": nbias3, "s_aug": s_aug, "x2": x2, "x2s": x2s, "xq": xq}


def _shuf_rows(a, ncol):
    """[R, ncol] row-major -> [P, R*ncol/P] with row (t*P+p) at [p, t*ncol:]."""
    return np.ascontiguousarray(
        a.reshape(T, P, ncol).transpose(1, 0, 2).reshape(P, T * ncol))


def _core_inputs(prep, c):
    cols = slice(c * R, (c + 1) * R)
    return {
        "xh_t": np.ascontiguousarray(prep["xh"][:, cols]),
        "xl_t": np.ascontiguousarray(prep["xl"][:, cols]),
        "x_shuf": _shuf_rows(prep["x"][cols, :], P),
        "x2in": _shuf_rows(prep["x2"][cols, :], 1),
        "x2s": _shuf_rows(prep["x2s"][cols, :], 1),
        "xq": _shuf_rows(prep["xq"][cols, :], 1),
        "nsh_t": prep["nsh"], "nsl_t": prep["nsl"],
        "nbias3": prep["nbias3"], "s_aug": prep["s_aug"],
    }


def kernel(**inputs):
    x = np.ascontiguousarray(np.asarray(inputs["defect_location"], dtype=np.float32))
    s = np.ascontiguousarray(np.asarray(inputs["defect_sites"], dtype=np.float32))
    rate = float(np.asarray(inputs["ricci_flow_rate"]).reshape(-1)[0])
    coh = float(np.asarray(inputs["cohesion"]).reshape(-1)[0])
    fric = float(np.asarray(inputs["friction_angle"]).reshape(-1)[0])
    tanf = float(np.float32(np.tan(np.float64(np.float32(fric)))))

    prep = _prep(x, s, rate, coh, tanf)
    key = (rate, coh, fric)
    if key not in _cache:
        _cache[key] = _build(rate, coh, tanf)
    nc = _cache[key]

    in_maps = [_core_inputs(prep, c) for c in range(NCORES)]
    res = bass_utils.run_bass_kernel_spmd(nc, in_maps,
                                          core_ids=list(range(NCORES)))
    outs = []
    for c in range(NCORES):
        b = res.results[c]["out"]          # [P, R] shuffled
        outs.append(b.reshape(P, T, P).transpose(1, 0, 2).reshape(R, P))
    return np.concatenate(outs, axis=0).astype(np.float32)


if __name__ == "__main__":
    import time
    x = np.load("/tmp/x.npy")
    s = np.load("/tmp/s.npy")
    rate, coh, fric = np.load("/tmp/scalars.npy")
    t0 = time.time()
    out = kernel(defect_location=x, defect_sites=s,
                 ricci_flow_rate=np.float32(rate), cohesion=np.float32(coh),
                 friction_angle=np.float32(fric))
    print("kernel wall:", time.time() - t0)
    exp = np.load("/tmp/expected.npy")
    err = np.abs(out - exp)
    rel = np.linalg.norm((out - exp).astype(np.float64)) / np.linalg.norm(exp.astype(np.float64))
    print("absmax err:", err.max(), "rel l2:", rel)
